# revision 13
# baseline (speedup 1.0000x reference)
"""Trainium2 Bass kernel for nn_ErosionLayer (B=8, W=512, ITERS=10).

Sharding: pure data parallel — one batch sample per NeuronCore (8 cores),
no collectives.  Each core runs the full 10-iteration erosion simulation
on its own 512x512 grid.

Key algorithmic mapping:
  * The bilinear gather (neighbor_height) has |displacement| <= 1 cell, so
    it reduces to the same separable 3x3 "hat" stencil that displace()
    uses: row weights hat(v - a), col weights hat(u - b) with
    hat(x) = max(0, 1 - |x|).  hat(-1)=relu(-x), hat(0)=1-|x|, hat(1)=relu(x).
  * sqrt(x) = exp(0.5*ln(x)) and 1/(mag+eps) = exp(-ln(mag+eps)) keep every
    transcendental inside the single `natural_log_exp_and_others` ACT table
    set (no table reloads).
  * The flat-gradient random-angle branch requires bit-exact fp32 equality
    of both central differences simultaneously (probability ~2^-46 per
    cell); it is statistically unreachable, so it is dropped and
    `random_gradient` is unused.

Memory layout per field: SBUF [NP, nblk, width] with row j = 4*p + m
(partition p, data block m).  Column halos live in the free dimension.
Fields read with row offsets (terrain, shifted displace accumulators) get
two extra halo BLOCKS (rows 4p-1 and 4p+4) so row shifts are free-dim
offsets; the halo blocks are refreshed with small SBUF->SBUF DMAs (DMA is
exempt from the partition-start alignment rule that compute engines have).
"""

import functools
import sys

import numpy as np

sys.path.insert(0, "/opt/trn_rl_repo")

W = 512
ITERS = 10
B = 8
N_CORES = 8
EPS = 1e-10
E8 = float(np.exp(-8.0))


def _scalars(rain_rate, evaporation_rate, min_height_delta, gravity,
             sediment_capacity_constant, dissolving_rate, deposition_rate,
             max_height_delta, alpha, wg):
    cell_width = 200.0 / wg
    return dict(
        RR=float(2.0 ** float(rain_rate)),
        GR=float(2.0 ** float(gravity)),
        MHD=float(np.float32(2.0 ** float(min_height_delta)) / np.float32(cell_width)),
        SCC=float(2.0 ** float(sediment_capacity_constant)),
        DEP=float(2.0 ** float(deposition_rate)),
        DIS=float(2.0 ** float(dissolving_rate)),
        EV=float(1.0 - 2.0 ** float(evaporation_rate)),
        MX=float(max_height_delta),
        ALPHA=float(alpha),
    )


def build_erosion(nc, tc, ctx, ins, outs, sc, wg, iters):
    """Emit the erosion program into TileContext tc.

    ins: dict of DRAM APs {'inp': [wg,wg], 'orig': [wg,wg], 'rain': [iters,wg,wg]}
    outs: {'out': [wg,wg]}
    sc: baked python-float scalars (see _scalars)
    """
    import concourse.bass as bass  # noqa: F401
    from concourse import mybir

    Alu = mybir.AluOpType
    Af = mybir.ActivationFunctionType

    NP = wg // 4          # partitions used
    IW = wg               # interior width
    SW = wg + 2           # +-1 col halo (col i at offset i+1)
    TW = wg + 4           # terrain width: cols -2..wg+1 (col i at offset i+2)

    f32 = mybir.dt.float32

    TT = nc.vector.tensor_tensor
    TSS = nc.vector.tensor_single_scalar
    TS2 = nc.vector.tensor_scalar
    STT = nc.vector.scalar_tensor_tensor
    CPY = nc.vector.tensor_copy

    def ACT(out, in_, func, bias=0.0, scale=1.0):
        nc.scalar.activation(out, in_, func, bias=bias, scale=scale)

    state = ctx.enter_context(tc.tile_pool(name="state", bufs=1))
    rain_pool = ctx.enter_context(tc.tile_pool(name="rain", bufs=1))
    work_pool = ctx.enter_context(tc.tile_pool(name="work", bufs=8))
    cpool = ctx.enter_context(tc.tile_pool(name="cpool", bufs=1))

    # Terrain: 6 blocks (row-halo blocks 0 and 5), TW wide.
    T = state.tile([NP, 6, TW], f32, tag="T")
    S = state.tile([NP, 4, SW], f32, tag="S")
    Wt = state.tile([NP, 4, SW], f32, tag="Wt")
    VS = state.tile([NP, 4, IW], f32, tag="VS")
    wgt = {}
    for nm in ("cwm", "cwc", "cwp", "rwm", "rwc", "rwp"):
        wgt[nm] = state.tile([NP, 4, SW], f32, tag=nm, name=nm)

    def w():
        return work_pool.tile([NP, 4, SW], f32, tag="w", name="w")

    def wi(t):
        return t[:, :, 0:IW]

    T_int = T[:, 1:5, 2:2 + IW]
    S_int = S[:, :, 1:1 + IW]
    Wt_int = Wt[:, :, 1:1 + IW]
    VS_int = VS[:, :, :]

    def halo2(t):
        # refresh +-1 col halo of an SW-wide field
        CPY(t[:, :, 0:1], t[:, :, SW - 2:SW - 1])
        CPY(t[:, :, SW - 1:SW], t[:, :, 1:2])

    def halo_T():
        # col halos on the data blocks, then row-halo blocks via DMA
        CPY(T[:, 1:5, 0:2], T[:, 1:5, TW - 4:TW - 2])
        CPY(T[:, 1:5, TW - 2:TW], T[:, 1:5, 2:4])
        # block 0 = row 4p-1 = partition p-1's last data block (block 4)
        nc.sync.dma_start(out=T[1:NP, 0:1, :], in_=T[0:NP - 1, 4:5, :])
        nc.sync.dma_start(out=T[0:1, 0:1, :], in_=T[NP - 1:NP, 4:5, :])
        # block 5 = row 4p+4 = partition p+1's first data block (block 1)
        nc.sync.dma_start(out=T[0:NP - 1, 5:6, :], in_=T[1:NP, 1:2, :])
        nc.sync.dma_start(out=T[NP - 1:NP, 5:6, :], in_=T[0:1, 1:2, :])

    def displace(x_full, out_int, eng_prod, eng_acc, cmnm):
        """out_int = displace(x) interior; x_full is SW-wide with valid halos.

        out[j,i] = sum_{k0,k1} (x*cw[k0]*rw[k1])[j-k1, i-k0].
        eng_prod runs the 12 product TTs, eng_acc the 8 accumulate TTs.
        """
        # Cm (k1=-1) is read at row j+1 -> needs top halo block 5.
        # Cp (k1=+1) is read at row j-1 -> needs bottom halo block 0.
        Cm = cpool.tile([NP, 6, IW], f32, tag="cm", name="cm")
        Cp = cpool.tile([NP, 6, IW], f32, tag="cp", name="cp")
        C0 = w()
        for k1, rwn, Cd in ((-1, "rwm", Cm[:, 1:5, :]), (0, "rwc", wi(C0)),
                            (1, "rwp", Cp[:, 1:5, :])):
            SR = w()
            eng_prod.tensor_tensor(SR[:], x_full, wgt[rwn][:], Alu.mult)
            PS = {}
            for k0, cwn in ((-1, "cwm"), (0, "cwc"), (1, "cwp")):
                PS[k0] = w()
                eng_prod.tensor_tensor(PS[k0][:], SR[:], wgt[cwn][:], Alu.mult)
            # C[i] = PS[-1][i+1] + PS[0][i] + PS[+1][i-1]; col i at offset i+1
            eng_acc.tensor_tensor(Cd, PS[-1][:, :, 2:2 + IW],
                                  PS[0][:, :, 1:1 + IW], Alu.add)
            eng_acc.tensor_tensor(Cd, Cd, PS[1][:, :, 0:IW], Alu.add)
        # row-halo blocks via DMA
        nc.sync.dma_start(out=Cm[0:NP - 1, 5:6, :], in_=Cm[1:NP, 1:2, :])
        nc.sync.dma_start(out=Cm[NP - 1:NP, 5:6, :], in_=Cm[0:1, 1:2, :])
        nc.sync.dma_start(out=Cp[1:NP, 0:1, :], in_=Cp[0:NP - 1, 4:5, :])
        nc.sync.dma_start(out=Cp[0:1, 0:1, :], in_=Cp[NP - 1:NP, 4:5, :])
        # out[j] = Cm[j+1] + C0[j] + Cp[j-1]
        eng_acc.tensor_tensor(out_int, Cm[:, 2:6, :], wi(C0), Alu.add)
        eng_acc.tensor_tensor(out_int, out_int, Cp[:, 0:4, :], Alu.add)

    # ---------------- init ----------------
    orig_b = w()
    nc.sync.dma_start(
        out=wi(orig_b), in_=ins["orig"].rearrange("(p m) c -> p m c", p=NP))
    inp_b = w()
    nc.sync.dma_start(
        out=wi(inp_b), in_=ins["inp"].rearrange("(p m) c -> p m c", p=NP))
    t0 = w()
    TSS(wi(t0), wi(inp_b), sc["ALPHA"], Alu.mult)
    STT(T_int, wi(orig_b), 1.0 - sc["ALPHA"], wi(t0), Alu.mult, Alu.add)
    TS2(T_int, T_int, 0.5, 0.5, Alu.mult, Alu.add)
    halo_T()
    nc.vector.memset(S[:], 0.0)
    nc.vector.memset(Wt[:], 0.0)
    nc.vector.memset(VS[:], 0.0)

    rain_r = ins["rain"].rearrange("t (p m) c -> t p m c", p=NP)

    # ---------------- iterations ----------------
    for t in range(iters):
        rain_b = rain_pool.tile([NP, 4, IW], f32, tag="rain", name="rain_b")
        nc.sync.dma_start(out=rain_b[:], in_=rain_r[t])

        # gradient (T blocks: data at 1..4; row offset a -> blocks 1+a..5+a)
        DyR = w()
        TT(wi(DyR), T[:, 0:4, 2:2 + IW], T[:, 2:6, 2:2 + IW], Alu.subtract)
        DxR = w()
        TT(wi(DxR), T[:, 1:5, 1:1 + IW], T[:, 1:5, 3:3 + IW], Alu.subtract)
        sqy = w()
        ACT(wi(sqy), wi(DyR), Af.Square, scale=0.5)
        sqx = w()
        ACT(wi(sqx), wi(DxR), Af.Square, scale=0.5)
        s2 = w()
        STT(wi(s2), wi(sqx), 1e-30, wi(sqy), Alu.max, Alu.add)
        lns = w()
        ACT(wi(lns), wi(s2), Af.Ln)
        mag = w()
        ACT(wi(mag), wi(lns), Af.Exp, scale=0.5)
        lnm = w()
        ACT(wi(lnm), wi(mag), Af.Ln, bias=EPS)
        rc = w()
        ACT(wi(rc), wi(lnm), Af.Exp, scale=-1.0)
        gx = w()
        STT(wi(gx), wi(DxR), 0.5, wi(rc), Alu.mult, Alu.mult)
        gy = w()
        STT(wi(gy), wi(DyR), 0.5, wi(rc), Alu.mult, Alu.mult)
        # u = gy drives column weights, v = gx drives row weights (the
        # reference swaps gradient components before sampling/displacing)
        for u_t, pre in ((gy, "c"), (gx, "r")):
            m_i = wgt[pre + "wm"][:, :, 1:1 + IW]
            p_i = wgt[pre + "wp"][:, :, 1:1 + IW]
            c_i = wgt[pre + "wc"][:, :, 1:1 + IW]
            ACT(m_i, wi(u_t), Af.Relu, scale=-1.0)
            ACT(p_i, wi(u_t), Af.Relu)
            tw_ = w()
            TT(wi(tw_), m_i, p_i, Alu.add)
            TS2(c_i, wi(tw_), -1.0, 1.0, Alu.mult, Alu.add)
        for nm in ("cwm", "cwc", "cwp", "rwm", "rwc", "rwp"):
            halo2(wgt[nm])

        # gather: nb = sum_a rw[a] * (sum_b cw[b] * T[j+a, i+b])
        nb = w()
        first_a = True
        for rwn, a in (("rwm", -1), ("rwc", 0), ("rwp", 1)):
            G = w()
            tmp = w()
            TT(wi(G), wgt["cwm"][:, :, 1:1 + IW],
               T[:, 1 + a:5 + a, 1:1 + IW], Alu.mult)
            TT(wi(tmp), wgt["cwc"][:, :, 1:1 + IW],
               T[:, 1 + a:5 + a, 2:2 + IW], Alu.mult)
            TT(wi(G), wi(G), wi(tmp), Alu.add)
            TT(wi(tmp), wgt["cwp"][:, :, 1:1 + IW],
               T[:, 1 + a:5 + a, 3:3 + IW], Alu.mult)
            TT(wi(G), wi(G), wi(tmp), Alu.add)
            if first_a:
                TT(wi(nb), wgt[rwn][:, :, 1:1 + IW], wi(G), Alu.mult)
                first_a = False
            else:
                TT(wi(tmp), wgt[rwn][:, :, 1:1 + IW], wi(G), Alu.mult)
                TT(wi(nb), wi(nb), wi(tmp), Alu.add)
        hd = w()
        TT(wi(hd), T_int, wi(nb), Alu.subtract)

        # velocity (carry VS = velocity^2; V = exp(0.5 ln VS))
        vsn = w()
        STT(wi(vsn), wi(hd), sc["GR"], VS_int, Alu.mult, Alu.add)
        rz5 = w()
        ACT(wi(rz5), wi(vsn), Af.Relu, bias=-EPS)
        t5 = w()
        ACT(wi(t5), wi(vsn), Af.Relu, scale=-1.0, bias=EPS + 8.0)
        e5 = w()
        ACT(wi(e5), wi(t5), Af.Exp, scale=-1.0)
        m5 = w()
        TSS(wi(m5), wi(e5), E8, Alu.min)
        STT(VS_int, wi(m5), EPS, wi(rz5), Alu.add, Alu.add)
        lnv = w()
        ACT(wi(lnv), VS_int, Af.Ln)
        vel = w()
        ACT(wi(vel), wi(lnv), Af.Exp, scale=0.5)

        # water += rain * 2^rain_rate   (rain >= 0 so the relu is identity;
        # deferred to here so the single-buffered rain DMA hides)
        STT(Wt_int, rain_b[:], sc["RR"], Wt_int, Alu.mult, Alu.add)
        halo2(Wt)

        # new_hd = soft_floor(hd, MHD)
        rz6 = w()
        ACT(wi(rz6), wi(hd), Af.Relu, bias=-sc["MHD"])
        t6 = w()
        ACT(wi(t6), wi(hd), Af.Relu, scale=-1.0, bias=sc["MHD"] + 8.0)
        e6 = w()
        ACT(wi(e6), wi(t6), Af.Exp, scale=-1.0)
        m6 = w()
        TSS(wi(m6), wi(e6), E8, Alu.min)
        nhd = w()
        STT(wi(nhd), wi(m6), sc["MHD"], wi(rz6), Alu.add, Alu.add)

        # sediment capacity
        t7 = w()
        TT(wi(t7), wi(nhd), wi(vel), Alu.mult)
        scap = w()
        STT(wi(scap), wi(t7), sc["SCC"], Wt_int, Alu.mult, Alu.mult)

        # branch coefficients
        ftb = w()
        TSS(wi(ftb), wi(hd), 0.0, Alu.is_lt)
        stb = w()
        TSS(wi(stb), wi(hd), sc["MX"], Alu.is_gt)
        fs = w()
        TT(wi(fs), wi(ftb), wi(stb), Alu.add)
        coef = w()
        TS2(wi(coef), wi(fs), -1.0, 1.0, Alu.mult, Alu.add)
        t9 = w()
        TS2(wi(t9), wi(hd), -1.0, sc["MX"], Alu.mult, Alu.add)
        second = w()
        TT(wi(second), wi(stb), wi(t9), Alu.mult)

        # first
        mint = w()
        ACT(wi(mint), wi(hd), Af.Relu, scale=-1.0)
        z3 = w()
        TT(wi(z3), wi(mint), S_int, Alu.subtract)
        rz3 = w()
        ACT(wi(rz3), wi(z3), Af.Relu)
        t8 = w()
        ACT(wi(t8), wi(z3), Af.Relu, scale=-1.0, bias=8.0)
        e8 = w()
        ACT(wi(e8), wi(t8), Af.Exp, scale=-1.0)
        m8 = w()
        TSS(wi(m8), wi(e8), E8, Alu.min)
        q = w()
        TT(wi(q), wi(rz3), wi(m8), Alu.add)
        first = w()
        TT(wi(first), wi(mint), wi(q), Alu.subtract)

        # third
        sdiff = w()
        TT(wi(sdiff), S_int, wi(scap), Alu.subtract)
        r1 = w()
        ACT(wi(r1), wi(sdiff), Af.Relu, scale=sc["DEP"])
        r2 = w()
        ACT(wi(r2), wi(sdiff), Af.Relu, scale=-sc["DIS"])
        t10 = w()
        TT(wi(t10), wi(r1), wi(r2), Alu.subtract)
        third = w()
        TT(wi(third), wi(coef), wi(t10), Alu.mult)

        # deposited = soft_floor(first+second+third, -relu(hd))
        m4 = w()
        ACT(wi(m4), wi(hd), Af.Relu)
        x4 = w()
        TT(wi(x4), wi(first), wi(second), Alu.add)
        TT(wi(x4), wi(x4), wi(third), Alu.add)
        z4 = w()
        TT(wi(z4), wi(x4), wi(m4), Alu.add)
        rz4 = w()
        ACT(wi(rz4), wi(z4), Af.Relu)
        t11 = w()
        ACT(wi(t11), wi(z4), Af.Relu, scale=-1.0, bias=8.0)
        e11 = w()
        ACT(wi(e11), wi(t11), Af.Exp, scale=-1.0)
        m11 = w()
        TSS(wi(m11), wi(e11), E8, Alu.min)
        dep0 = w()
        TT(wi(dep0), wi(rz4), wi(m11), Alu.add)
        depo = w()
        TT(wi(depo), wi(dep0), wi(m4), Alu.subtract)

        # state updates
        TT(S_int, S_int, wi(depo), Alu.subtract)
        halo2(S)
        TT(T_int, T_int, wi(depo), Alu.add)
        halo_T()

        # displace sediment, then water (water pre-scaled by 1-2^evap).
        # Water displace runs fully on GPSIMD (its result is only needed
        # next iteration); sediment products on DVE, accumulates on GPSIMD.
        displace(S[:], S_int, nc.vector, nc.gpsimd, "cs")
        Wtk = w()
        TSS(Wtk[:], Wt[:], sc["EV"], Alu.mult)
        displace(Wtk[:], Wt_int, nc.vector, nc.gpsimd, "cw")

    # ---------------- output ----------------
    ob = w()
    TS2(wi(ob), T_int, 2.0, -1.0, Alu.mult, Alu.add)
    nc.sync.dma_start(
        out=outs["out"].rearrange("(p m) c -> p m c", p=NP), in_=wi(ob))


@functools.lru_cache(maxsize=2)
def _compiled(scalar_key, wg, iters):
    from contextlib import ExitStack

    import concourse.tile as tile
    from concourse import bacc, mybir

    sc = dict(scalar_key)
    nc = bacc.Bacc("TRN2", target_bir_lowering=False, debug=False,
                   num_devices=N_CORES)
    f32 = mybir.dt.float32
    # Pre-register const APs for every activation bias value we use.
    for i, v in enumerate([EPS, -EPS, EPS + 8.0, 8.0, -sc["MHD"],
                           sc["MHD"] + 8.0]):
        v = float(v)
        if (f32, v) not in nc.const_aps.aps:
            ct = nc.alloc_sbuf_tensor(f"constf32_{i}", [128, 1], f32)
            nc.gpsimd.memset(ct.ap(), v)
            nc.const_aps.aps[(f32, v)] = ct.ap()
    nc.all_engine_barrier()
    # Force every activation function into the one table set that contains
    # them all (natural_log_exp_and_others: exp, ln, relu, square, ...) so
    # the compiler never inserts mid-kernel ACT_TABLE_LOAD switches.
    try:
        from concourse.hw_specs import get_activation_tables

        tbl = get_activation_tables(nc.m.arch)
        keep = {mybir.ActivationFunctionType.Exp, mybir.ActivationFunctionType.Ln,
                mybir.ActivationFunctionType.Relu,
                mybir.ActivationFunctionType.Square}
        if "natural_log_exp_and_others" in tbl and keep <= tbl[
                "natural_log_exp_and_others"]:
            for name, fns in tbl.items():
                if name != "natural_log_exp_and_others":
                    fns -= keep
    except Exception:
        pass
    inp = nc.dram_tensor("inp", [wg, wg], f32, kind="ExternalInput")
    orig = nc.dram_tensor("orig", [wg, wg], f32, kind="ExternalInput")
    rain = nc.dram_tensor("rain", [iters, wg, wg], f32, kind="ExternalInput")
    out = nc.dram_tensor("out", [wg, wg], f32, kind="ExternalOutput")
    ins = {"inp": inp.ap(), "orig": orig.ap(), "rain": rain.ap()}
    outs = {"out": out.ap()}
    with ExitStack() as ctx:
        tc = ctx.enter_context(tile.TileContext(nc))
        build_erosion(nc, tc, ctx, ins, outs, sc, wg, iters)
    nc.compile()
    return nc


def kernel(**inputs):
    from concourse.bass_utils import run_bass_kernel_spmd

    it = np.ascontiguousarray(np.asarray(inputs["input_terrain"], np.float32))
    ot = np.ascontiguousarray(np.asarray(inputs["original_terrain"], np.float32))
    rain = np.ascontiguousarray(
        np.asarray(inputs["random_rainfall"], np.float32)[0])  # [ITERS, W, W]
    sc = _scalars(
        inputs["rain_rate"], inputs["evaporation_rate"],
        inputs["min_height_delta"], inputs["gravity"],
        inputs["sediment_capacity_constant"], inputs["dissolving_rate"],
        inputs["deposition_rate"], inputs["max_height_delta"],
        inputs["alpha"], W)
    nc = _compiled(tuple(sorted(sc.items())), W, ITERS)
    in_maps = [
        {"inp": it[c], "orig": ot[c], "rain": rain} for c in range(B)
    ]
    res = run_bass_kernel_spmd(nc, in_maps, core_ids=list(range(N_CORES)))
    out = np.stack([res.results[c]["out"] for c in range(B)])[:, None]
    return out.astype(np.float32)


if __name__ == "__main__":
    # smoke build
    sc = _scalars(-6.0388, -5.643, -10.965, 4.906, 5.643, -2.0, -4.321,
                  -8.965, 0.0, W)
    nc = _compiled(tuple(sorted(sc.items())), W, ITERS)
    print("built ok:",
          sum(len(b.instructions) for b in nc.main_func.blocks), "instructions")


# revision 18
# speedup vs baseline: 696.9846x; 696.9846x over previous
"""Trainium2 Bass kernel for nn_ErosionLayer (B=8, W=512, ITERS=10).

Sharding: pure data parallel — one batch sample per NeuronCore (8 cores),
no collectives.  Each core runs the full 10-iteration erosion simulation
on its own 512x512 grid.

Key algorithmic mapping:
  * The bilinear gather (neighbor_height) has |displacement| <= 1 cell, so
    it reduces to the same separable 3x3 "hat" stencil that displace()
    uses: row weights hat(v - a), col weights hat(u - b) with
    hat(x) = max(0, 1 - |x|).  hat(-1)=relu(-x), hat(0)=1-|x|, hat(1)=relu(x).
  * sqrt(x) = exp(0.5*ln(x)) and 1/(mag+eps) = exp(-ln(mag+eps)) keep every
    transcendental inside the single `natural_log_exp_and_others` ACT table
    set (no table reloads).
  * The flat-gradient random-angle branch requires bit-exact fp32 equality
    of both central differences simultaneously (probability ~2^-46 per
    cell); it is statistically unreachable, so it is dropped and
    `random_gradient` is unused.

Memory layout per field: SBUF [NP, nblk, width] with row j = 4*p + m
(partition p, data block m).  Column halos live in the free dimension.
Fields read with row offsets (terrain, shifted displace accumulators) get
two extra halo BLOCKS (rows 4p-1 and 4p+4) so row shifts are free-dim
offsets; the halo blocks are refreshed with small SBUF->SBUF DMAs (DMA is
exempt from the partition-start alignment rule that compute engines have).
"""

import functools
import sys

import numpy as np

sys.path.insert(0, "/opt/trn_rl_repo")

W = 512
ITERS = 10
B = 8
N_CORES = 8
EPS = 1e-10
E8 = float(np.exp(-8.0))


def _scalars(rain_rate, evaporation_rate, min_height_delta, gravity,
             sediment_capacity_constant, dissolving_rate, deposition_rate,
             max_height_delta, alpha, wg):
    cell_width = 200.0 / wg
    return dict(
        RR=float(2.0 ** float(rain_rate)),
        GR=float(2.0 ** float(gravity)),
        MHD=float(np.float32(2.0 ** float(min_height_delta)) / np.float32(cell_width)),
        SCC=float(2.0 ** float(sediment_capacity_constant)),
        DEP=float(2.0 ** float(deposition_rate)),
        DIS=float(2.0 ** float(dissolving_rate)),
        EV=float(1.0 - 2.0 ** float(evaporation_rate)),
        MX=float(max_height_delta),
        ALPHA=float(alpha),
    )


def build_erosion(nc, tc, ctx, ins, outs, sc, wg, iters):
    """Emit the erosion program into TileContext tc.

    ins: dict of DRAM APs {'inp': [wg,wg], 'orig': [wg,wg], 'rain': [iters,wg,wg]}
    outs: {'out': [wg,wg]}
    sc: baked python-float scalars (see _scalars)
    """
    import concourse.bass as bass  # noqa: F401
    from concourse import mybir

    Alu = mybir.AluOpType
    Af = mybir.ActivationFunctionType

    NP = wg // 4          # partitions used
    IW = wg               # interior width
    SW = wg + 2           # +-1 col halo (col i at offset i+1)
    TW = wg + 4           # terrain width: cols -2..wg+1 (col i at offset i+2)

    f32 = mybir.dt.float32

    TT = nc.vector.tensor_tensor
    TSS = nc.vector.tensor_single_scalar
    TS2 = nc.vector.tensor_scalar
    STT = nc.vector.scalar_tensor_tensor
    CPY = nc.vector.tensor_copy

    def ACT(out, in_, func, bias=0.0, scale=1.0):
        nc.scalar.activation(out, in_, func, bias=bias, scale=scale)

    state = ctx.enter_context(tc.tile_pool(name="state", bufs=1))
    rain_pool = ctx.enter_context(tc.tile_pool(name="rain", bufs=1))
    work_pool = ctx.enter_context(tc.tile_pool(name="work", bufs=8))
    cpool = ctx.enter_context(tc.tile_pool(name="cpool", bufs=1))

    # Terrain: 6 blocks (row-halo blocks 0 and 5), TW wide.
    T = state.tile([NP, 6, TW], f32, tag="T")
    S = state.tile([NP, 4, SW], f32, tag="S")
    Wt = state.tile([NP, 4, SW], f32, tag="Wt")
    VS = state.tile([NP, 4, IW], f32, tag="VS")
    wgt = {}
    for nm in ("cwm", "cwc", "cwp", "rwm", "rwc", "rwp"):
        wgt[nm] = state.tile([NP, 4, SW], f32, tag=nm, name=nm)

    def w():
        return work_pool.tile([NP, 4, SW], f32, tag="w", name="w")

    def wi(t):
        return t[:, :, 0:IW]

    T_int = T[:, 1:5, 2:2 + IW]
    S_int = S[:, :, 1:1 + IW]
    Wt_int = Wt[:, :, 1:1 + IW]
    VS_int = VS[:, :, :]

    def halo2(t):
        # refresh +-1 col halo of an SW-wide field
        CPY(t[:, :, 0:1], t[:, :, SW - 2:SW - 1])
        CPY(t[:, :, SW - 1:SW], t[:, :, 1:2])

    def halo_T():
        # col halos on the data blocks, then row-halo blocks via DMA
        CPY(T[:, 1:5, 0:2], T[:, 1:5, TW - 4:TW - 2])
        CPY(T[:, 1:5, TW - 2:TW], T[:, 1:5, 2:4])
        # block 0 = row 4p-1 = partition p-1's last data block (block 4)
        nc.sync.dma_start(out=T[1:NP, 0:1, :], in_=T[0:NP - 1, 4:5, :])
        nc.sync.dma_start(out=T[0:1, 0:1, :], in_=T[NP - 1:NP, 4:5, :])
        # block 5 = row 4p+4 = partition p+1's first data block (block 1)
        nc.sync.dma_start(out=T[0:NP - 1, 5:6, :], in_=T[1:NP, 1:2, :])
        nc.sync.dma_start(out=T[NP - 1:NP, 5:6, :], in_=T[0:1, 1:2, :])

    def displace(x_full, out_int, eng_prod, eng_acc, cmnm):
        """out_int = displace(x) interior; x_full is SW-wide with valid halos.

        out[j,i] = sum_{k0,k1} (x*cw[k0]*rw[k1])[j-k1, i-k0].
        eng_prod runs the 12 product TTs, eng_acc the 8 accumulate TTs.
        """
        # Cm (k1=-1) is read at row j+1 -> needs top halo block 5.
        # Cp (k1=+1) is read at row j-1 -> needs bottom halo block 0.
        Cm = cpool.tile([NP, 6, IW], f32, tag="cm", name="cm")
        Cp = cpool.tile([NP, 6, IW], f32, tag="cp", name="cp")
        C0 = w()
        for k1, rwn, Cd in ((-1, "rwm", Cm[:, 1:5, :]), (0, "rwc", wi(C0)),
                            (1, "rwp", Cp[:, 1:5, :])):
            SR = w()
            eng_prod.tensor_tensor(SR[:], x_full, wgt[rwn][:], Alu.mult)
            PS = {}
            for k0, cwn in ((-1, "cwm"), (0, "cwc"), (1, "cwp")):
                PS[k0] = w()
                eng_prod.tensor_tensor(PS[k0][:], SR[:], wgt[cwn][:], Alu.mult)
            # C[i] = PS[-1][i+1] + PS[0][i] + PS[+1][i-1]; col i at offset i+1
            eng_acc.tensor_tensor(Cd, PS[-1][:, :, 2:2 + IW],
                                  PS[0][:, :, 1:1 + IW], Alu.add)
            eng_acc.tensor_tensor(Cd, Cd, PS[1][:, :, 0:IW], Alu.add)
        # row-halo blocks via DMA
        nc.sync.dma_start(out=Cm[0:NP - 1, 5:6, :], in_=Cm[1:NP, 1:2, :])
        nc.sync.dma_start(out=Cm[NP - 1:NP, 5:6, :], in_=Cm[0:1, 1:2, :])
        nc.sync.dma_start(out=Cp[1:NP, 0:1, :], in_=Cp[0:NP - 1, 4:5, :])
        nc.sync.dma_start(out=Cp[0:1, 0:1, :], in_=Cp[NP - 1:NP, 4:5, :])
        # out[j] = Cm[j+1] + C0[j] + Cp[j-1]
        eng_acc.tensor_tensor(out_int, Cm[:, 2:6, :], wi(C0), Alu.add)
        eng_acc.tensor_tensor(out_int, out_int, Cp[:, 0:4, :], Alu.add)

    # ---------------- init ----------------
    orig_b = w()
    nc.sync.dma_start(
        out=wi(orig_b), in_=ins["orig"].rearrange("(p m) c -> p m c", p=NP))
    inp_b = w()
    nc.sync.dma_start(
        out=wi(inp_b), in_=ins["inp"].rearrange("(p m) c -> p m c", p=NP))
    t0 = w()
    TSS(wi(t0), wi(inp_b), sc["ALPHA"], Alu.mult)
    STT(T_int, wi(orig_b), 1.0 - sc["ALPHA"], wi(t0), Alu.mult, Alu.add)
    TS2(T_int, T_int, 0.5, 0.5, Alu.mult, Alu.add)
    halo_T()
    nc.vector.memset(S[:], 0.0)
    nc.vector.memset(Wt[:], 0.0)
    nc.vector.memset(VS[:], 0.0)

    rain_r = ins["rain"].rearrange("t (p m) c -> t p m c", p=NP)

    # ---------------- iterations ----------------
    for t in range(iters):
        rain_b = rain_pool.tile([NP, 4, IW], f32, tag="rain", name="rain_b")
        nc.sync.dma_start(out=rain_b[:], in_=rain_r[t])

        # gradient (T blocks: data at 1..4; row offset a -> blocks 1+a..5+a)
        DyR = w()
        TT(wi(DyR), T[:, 0:4, 2:2 + IW], T[:, 2:6, 2:2 + IW], Alu.subtract)
        DxR = w()
        TT(wi(DxR), T[:, 1:5, 1:1 + IW], T[:, 1:5, 3:3 + IW], Alu.subtract)
        sqy = w()
        ACT(wi(sqy), wi(DyR), Af.Square, scale=0.5)
        sqx = w()
        ACT(wi(sqx), wi(DxR), Af.Square, scale=0.5)
        s2 = w()
        STT(wi(s2), wi(sqx), 1e-30, wi(sqy), Alu.max, Alu.add)
        lns = w()
        ACT(wi(lns), wi(s2), Af.Ln)
        mag = w()
        ACT(wi(mag), wi(lns), Af.Exp, scale=0.5)
        lnm = w()
        ACT(wi(lnm), wi(mag), Af.Ln, bias=EPS)
        rc = w()
        ACT(wi(rc), wi(lnm), Af.Exp, scale=-1.0)
        gx = w()
        STT(wi(gx), wi(DxR), 0.5, wi(rc), Alu.mult, Alu.mult)
        gy = w()
        STT(wi(gy), wi(DyR), 0.5, wi(rc), Alu.mult, Alu.mult)
        # u = gy drives column weights, v = gx drives row weights (the
        # reference swaps gradient components before sampling/displacing)
        for u_t, pre in ((gy, "c"), (gx, "r")):
            m_i = wgt[pre + "wm"][:, :, 1:1 + IW]
            p_i = wgt[pre + "wp"][:, :, 1:1 + IW]
            c_i = wgt[pre + "wc"][:, :, 1:1 + IW]
            ACT(m_i, wi(u_t), Af.Relu, scale=-1.0)
            ACT(p_i, wi(u_t), Af.Relu)
            tw_ = w()
            TT(wi(tw_), m_i, p_i, Alu.add)
            TS2(c_i, wi(tw_), -1.0, 1.0, Alu.mult, Alu.add)
        for nm in ("cwm", "cwc", "cwp", "rwm", "rwc", "rwp"):
            halo2(wgt[nm])

        # gather: nb = sum_a rw[a] * (sum_b cw[b] * T[j+a, i+b])
        nb = w()
        first_a = True
        for rwn, a in (("rwm", -1), ("rwc", 0), ("rwp", 1)):
            G = w()
            tmp = w()
            tmp2 = w()
            TT(wi(G), wgt["cwm"][:, :, 1:1 + IW],
               T[:, 1 + a:5 + a, 1:1 + IW], Alu.mult)
            TT(wi(tmp), wgt["cwc"][:, :, 1:1 + IW],
               T[:, 1 + a:5 + a, 2:2 + IW], Alu.mult)
            TT(wi(G), wi(G), wi(tmp), Alu.add)
            TT(wi(tmp2), wgt["cwp"][:, :, 1:1 + IW],
               T[:, 1 + a:5 + a, 3:3 + IW], Alu.mult)
            TT(wi(G), wi(G), wi(tmp2), Alu.add)
            if first_a:
                TT(wi(nb), wgt[rwn][:, :, 1:1 + IW], wi(G), Alu.mult)
                first_a = False
            else:
                TT(wi(tmp), wgt[rwn][:, :, 1:1 + IW], wi(G), Alu.mult)
                TT(wi(nb), wi(nb), wi(tmp), Alu.add)
        hd = w()
        TT(wi(hd), T_int, wi(nb), Alu.subtract)

        # velocity (carry VS = velocity^2; V = exp(0.5 ln VS))
        vsn = w()
        STT(wi(vsn), wi(hd), sc["GR"], VS_int, Alu.mult, Alu.add)
        rz5 = w()
        ACT(wi(rz5), wi(vsn), Af.Relu, bias=-EPS)
        t5 = w()
        ACT(wi(t5), wi(vsn), Af.Relu, scale=-1.0, bias=EPS + 8.0)
        e5 = w()
        ACT(wi(e5), wi(t5), Af.Exp, scale=-1.0)
        m5 = w()
        TSS(wi(m5), wi(e5), E8, Alu.min)
        STT(VS_int, wi(m5), EPS, wi(rz5), Alu.add, Alu.add)
        lnv = w()
        ACT(wi(lnv), VS_int, Af.Ln)
        vel = w()
        ACT(wi(vel), wi(lnv), Af.Exp, scale=0.5)

        # water += rain * 2^rain_rate   (rain >= 0 so the relu is identity;
        # deferred to here so the single-buffered rain DMA hides)
        STT(Wt_int, rain_b[:], sc["RR"], Wt_int, Alu.mult, Alu.add)
        halo2(Wt)

        # new_hd = soft_floor(hd, MHD)
        rz6 = w()
        ACT(wi(rz6), wi(hd), Af.Relu, bias=-sc["MHD"])
        t6 = w()
        ACT(wi(t6), wi(hd), Af.Relu, scale=-1.0, bias=sc["MHD"] + 8.0)
        e6 = w()
        ACT(wi(e6), wi(t6), Af.Exp, scale=-1.0)
        m6 = w()
        TSS(wi(m6), wi(e6), E8, Alu.min)
        nhd = w()
        STT(wi(nhd), wi(m6), sc["MHD"], wi(rz6), Alu.add, Alu.add)

        # sediment capacity
        t7 = w()
        TT(wi(t7), wi(nhd), wi(vel), Alu.mult)
        scap = w()
        STT(wi(scap), wi(t7), sc["SCC"], Wt_int, Alu.mult, Alu.mult)

        # branch coefficients
        ftb = w()
        TSS(wi(ftb), wi(hd), 0.0, Alu.is_lt)
        stb = w()
        TSS(wi(stb), wi(hd), sc["MX"], Alu.is_gt)
        fs = w()
        TT(wi(fs), wi(ftb), wi(stb), Alu.add)
        coef = w()
        TS2(wi(coef), wi(fs), -1.0, 1.0, Alu.mult, Alu.add)
        t9 = w()
        TS2(wi(t9), wi(hd), -1.0, sc["MX"], Alu.mult, Alu.add)
        second = w()
        TT(wi(second), wi(stb), wi(t9), Alu.mult)

        # first
        mint = w()
        ACT(wi(mint), wi(hd), Af.Relu, scale=-1.0)
        z3 = w()
        TT(wi(z3), wi(mint), S_int, Alu.subtract)
        rz3 = w()
        ACT(wi(rz3), wi(z3), Af.Relu)
        t8 = w()
        ACT(wi(t8), wi(z3), Af.Relu, scale=-1.0, bias=8.0)
        e8 = w()
        ACT(wi(e8), wi(t8), Af.Exp, scale=-1.0)
        m8 = w()
        TSS(wi(m8), wi(e8), E8, Alu.min)
        q = w()
        TT(wi(q), wi(rz3), wi(m8), Alu.add)
        first = w()
        TT(wi(first), wi(mint), wi(q), Alu.subtract)

        # third
        sdiff = w()
        TT(wi(sdiff), S_int, wi(scap), Alu.subtract)
        r1 = w()
        ACT(wi(r1), wi(sdiff), Af.Relu, scale=sc["DEP"])
        r2 = w()
        ACT(wi(r2), wi(sdiff), Af.Relu, scale=-sc["DIS"])
        t10 = w()
        TT(wi(t10), wi(r1), wi(r2), Alu.subtract)
        third = w()
        TT(wi(third), wi(coef), wi(t10), Alu.mult)

        # deposited = soft_floor(first+second+third, -relu(hd))
        m4 = w()
        ACT(wi(m4), wi(hd), Af.Relu)
        x4 = w()
        TT(wi(x4), wi(first), wi(second), Alu.add)
        TT(wi(x4), wi(x4), wi(third), Alu.add)
        z4 = w()
        TT(wi(z4), wi(x4), wi(m4), Alu.add)
        rz4 = w()
        ACT(wi(rz4), wi(z4), Af.Relu)
        t11 = w()
        ACT(wi(t11), wi(z4), Af.Relu, scale=-1.0, bias=8.0)
        e11 = w()
        ACT(wi(e11), wi(t11), Af.Exp, scale=-1.0)
        m11 = w()
        TSS(wi(m11), wi(e11), E8, Alu.min)
        dep0 = w()
        TT(wi(dep0), wi(rz4), wi(m11), Alu.add)
        depo = w()
        TT(wi(depo), wi(dep0), wi(m4), Alu.subtract)

        # state updates
        TT(S_int, S_int, wi(depo), Alu.subtract)
        halo2(S)
        TT(T_int, T_int, wi(depo), Alu.add)
        halo_T()

        # displace sediment, then water (water pre-scaled by 1-2^evap).
        # Water displace runs fully on GPSIMD (its result is only needed
        # next iteration); sediment products on DVE, accumulates on GPSIMD.
        displace(S[:], S_int, nc.vector, nc.gpsimd, "cs")
        Wtk = w()
        TSS(Wtk[:], Wt[:], sc["EV"], Alu.mult)
        displace(Wtk[:], Wt_int, nc.vector, nc.gpsimd, "cw")

    # ---------------- output ----------------
    ob = w()
    TS2(wi(ob), T_int, 2.0, -1.0, Alu.mult, Alu.add)
    nc.sync.dma_start(
        out=outs["out"].rearrange("(p m) c -> p m c", p=NP), in_=wi(ob))


@functools.lru_cache(maxsize=2)
def _compiled(scalar_key, wg, iters):
    from contextlib import ExitStack

    import concourse.tile as tile
    from concourse import bacc, mybir

    sc = dict(scalar_key)
    nc = bacc.Bacc("TRN2", target_bir_lowering=False, debug=False,
                   num_devices=N_CORES)
    f32 = mybir.dt.float32
    # Pre-register const APs for every activation bias value we use.
    for i, v in enumerate([EPS, -EPS, EPS + 8.0, 8.0, -sc["MHD"],
                           sc["MHD"] + 8.0]):
        v = float(v)
        if (f32, v) not in nc.const_aps.aps:
            ct = nc.alloc_sbuf_tensor(f"constf32_{i}", [128, 1], f32)
            nc.gpsimd.memset(ct.ap(), v)
            nc.const_aps.aps[(f32, v)] = ct.ap()
    nc.all_engine_barrier()
    # Force every activation function into the one table set that contains
    # them all (natural_log_exp_and_others: exp, ln, relu, square, ...) so
    # the compiler never inserts mid-kernel ACT_TABLE_LOAD switches.
    try:
        from concourse.hw_specs import get_activation_tables

        tbl = get_activation_tables(nc.m.arch)
        keep = {mybir.ActivationFunctionType.Exp, mybir.ActivationFunctionType.Ln,
                mybir.ActivationFunctionType.Relu,
                mybir.ActivationFunctionType.Square}
        if "natural_log_exp_and_others" in tbl and keep <= tbl[
                "natural_log_exp_and_others"]:
            for name, fns in tbl.items():
                if name != "natural_log_exp_and_others":
                    fns -= keep
    except Exception:
        pass
    inp = nc.dram_tensor("inp", [wg, wg], f32, kind="ExternalInput")
    orig = nc.dram_tensor("orig", [wg, wg], f32, kind="ExternalInput")
    rain = nc.dram_tensor("rain", [iters, wg, wg], f32, kind="ExternalInput")
    out = nc.dram_tensor("out", [wg, wg], f32, kind="ExternalOutput")
    ins = {"inp": inp.ap(), "orig": orig.ap(), "rain": rain.ap()}
    outs = {"out": out.ap()}
    with ExitStack() as ctx:
        tc = ctx.enter_context(tile.TileContext(nc))
        build_erosion(nc, tc, ctx, ins, outs, sc, wg, iters)
    nc.compile()
    return nc


def kernel(**inputs):
    from concourse.bass_utils import run_bass_kernel_spmd

    it = np.ascontiguousarray(np.asarray(inputs["input_terrain"], np.float32))
    ot = np.ascontiguousarray(np.asarray(inputs["original_terrain"], np.float32))
    rain = np.ascontiguousarray(
        np.asarray(inputs["random_rainfall"], np.float32)[0])  # [ITERS, W, W]
    sc = _scalars(
        inputs["rain_rate"], inputs["evaporation_rate"],
        inputs["min_height_delta"], inputs["gravity"],
        inputs["sediment_capacity_constant"], inputs["dissolving_rate"],
        inputs["deposition_rate"], inputs["max_height_delta"],
        inputs["alpha"], W)
    nc = _compiled(tuple(sorted(sc.items())), W, ITERS)
    in_maps = [
        {"inp": it[c], "orig": ot[c], "rain": rain} for c in range(B)
    ]
    res = run_bass_kernel_spmd(nc, in_maps, core_ids=list(range(N_CORES)))
    out = np.stack([res.results[c]["out"] for c in range(B)])[:, None]
    return out.astype(np.float32)


if __name__ == "__main__":
    # smoke build
    sc = _scalars(-6.0388, -5.643, -10.965, 4.906, 5.643, -2.0, -4.321,
                  -8.965, 0.0, W)
    nc = _compiled(tuple(sorted(sc.items())), W, ITERS)
    print("built ok:",
          sum(len(b.instructions) for b in nc.main_func.blocks), "instructions")


# revision 19
# speedup vs baseline: 711.2915x; 1.0205x over previous
"""Trainium2 Bass kernel for nn_ErosionLayer (B=8, W=512, ITERS=10).

Sharding: pure data parallel — one batch sample per NeuronCore (8 cores),
no collectives.  Each core runs the full 10-iteration erosion simulation
on its own 512x512 grid.

Key algorithmic mapping:
  * The bilinear gather (neighbor_height) has |displacement| <= 1 cell, so
    it reduces to the same separable 3x3 "hat" stencil that displace()
    uses: row weights hat(v - a), col weights hat(u - b) with
    hat(x) = max(0, 1 - |x|).  hat(-1)=relu(-x), hat(0)=1-|x|, hat(1)=relu(x).
  * sqrt(x) = exp(0.5*ln(x)) and 1/(mag+eps) = exp(-ln(mag+eps)) keep every
    transcendental inside the single `natural_log_exp_and_others` ACT table
    set (no table reloads).
  * The flat-gradient random-angle branch requires bit-exact fp32 equality
    of both central differences simultaneously (probability ~2^-46 per
    cell); it is statistically unreachable, so it is dropped and
    `random_gradient` is unused.

Memory layout per field: SBUF [NP, nblk, width] with row j = 4*p + m
(partition p, data block m).  Column halos live in the free dimension.
Fields read with row offsets (terrain, shifted displace accumulators) get
two extra halo BLOCKS (rows 4p-1 and 4p+4) so row shifts are free-dim
offsets; the halo blocks are refreshed with small SBUF->SBUF DMAs (DMA is
exempt from the partition-start alignment rule that compute engines have).
"""

import functools
import sys

import numpy as np

sys.path.insert(0, "/opt/trn_rl_repo")

W = 512
ITERS = 10
B = 8
N_CORES = 8
EPS = 1e-10
E8 = float(np.exp(-8.0))


def _scalars(rain_rate, evaporation_rate, min_height_delta, gravity,
             sediment_capacity_constant, dissolving_rate, deposition_rate,
             max_height_delta, alpha, wg):
    cell_width = 200.0 / wg
    return dict(
        RR=float(2.0 ** float(rain_rate)),
        GR=float(2.0 ** float(gravity)),
        MHD=float(np.float32(2.0 ** float(min_height_delta)) / np.float32(cell_width)),
        SCC=float(2.0 ** float(sediment_capacity_constant)),
        DEP=float(2.0 ** float(deposition_rate)),
        DIS=float(2.0 ** float(dissolving_rate)),
        EV=float(1.0 - 2.0 ** float(evaporation_rate)),
        MX=float(max_height_delta),
        ALPHA=float(alpha),
    )


def build_erosion(nc, tc, ctx, ins, outs, sc, wg, iters):
    """Emit the erosion program into TileContext tc.

    ins: dict of DRAM APs {'inp': [wg,wg], 'orig': [wg,wg], 'rain': [iters,wg,wg]}
    outs: {'out': [wg,wg]}
    sc: baked python-float scalars (see _scalars)
    """
    import concourse.bass as bass  # noqa: F401
    from concourse import mybir

    Alu = mybir.AluOpType
    Af = mybir.ActivationFunctionType

    NP = wg // 4          # partitions used
    IW = wg               # interior width
    SW = wg + 2           # +-1 col halo (col i at offset i+1)
    TW = wg + 4           # terrain width: cols -2..wg+1 (col i at offset i+2)

    f32 = mybir.dt.float32

    TT = nc.vector.tensor_tensor
    TSS = nc.vector.tensor_single_scalar
    TS2 = nc.vector.tensor_scalar
    STT = nc.vector.scalar_tensor_tensor
    CPY = nc.vector.tensor_copy

    def ACT(out, in_, func, bias=0.0, scale=1.0):
        nc.scalar.activation(out, in_, func, bias=bias, scale=scale)

    state = ctx.enter_context(tc.tile_pool(name="state", bufs=1))
    rain_pool = ctx.enter_context(tc.tile_pool(name="rain", bufs=1))
    work_pool = ctx.enter_context(tc.tile_pool(name="work", bufs=10))
    cpool = ctx.enter_context(tc.tile_pool(name="cpool", bufs=1))

    # Terrain: 6 blocks (row-halo blocks 0 and 5), TW wide.
    T = state.tile([NP, 6, TW], f32, tag="T")
    S = state.tile([NP, 4, SW], f32, tag="S")
    Wt = state.tile([NP, 4, SW], f32, tag="Wt")
    VS = state.tile([NP, 4, IW], f32, tag="VS")
    wgt = {}
    for nm in ("cwm", "cwc", "cwp", "rwm", "rwc", "rwp"):
        wgt[nm] = state.tile([NP, 4, SW], f32, tag=nm, name=nm)

    def w():
        return work_pool.tile([NP, 4, SW], f32, tag="w", name="w")

    def wi(t):
        return t[:, :, 0:IW]

    T_int = T[:, 1:5, 2:2 + IW]
    S_int = S[:, :, 1:1 + IW]
    Wt_int = Wt[:, :, 1:1 + IW]
    VS_int = VS[:, :, :]

    def halo2(t):
        # refresh +-1 col halo of an SW-wide field
        CPY(t[:, :, 0:1], t[:, :, SW - 2:SW - 1])
        CPY(t[:, :, SW - 1:SW], t[:, :, 1:2])

    def halo_T():
        # col halos on the data blocks, then row-halo blocks via DMA
        CPY(T[:, 1:5, 0:2], T[:, 1:5, TW - 4:TW - 2])
        CPY(T[:, 1:5, TW - 2:TW], T[:, 1:5, 2:4])
        # block 0 = row 4p-1 = partition p-1's last data block (block 4)
        nc.sync.dma_start(out=T[1:NP, 0:1, :], in_=T[0:NP - 1, 4:5, :])
        nc.sync.dma_start(out=T[0:1, 0:1, :], in_=T[NP - 1:NP, 4:5, :])
        # block 5 = row 4p+4 = partition p+1's first data block (block 1)
        nc.sync.dma_start(out=T[0:NP - 1, 5:6, :], in_=T[1:NP, 1:2, :])
        nc.sync.dma_start(out=T[NP - 1:NP, 5:6, :], in_=T[0:1, 1:2, :])

    def displace(x_full, out_int, eng_prod, eng_acc, cmnm):
        """out_int = displace(x) interior; x_full is SW-wide with valid halos.

        out[j,i] = sum_{k0,k1} (x*cw[k0]*rw[k1])[j-k1, i-k0].
        eng_prod runs the 12 product TTs, eng_acc the 8 accumulate TTs.
        """
        # Cm (k1=-1) is read at row j+1 -> needs top halo block 5.
        # Cp (k1=+1) is read at row j-1 -> needs bottom halo block 0.
        Cm = cpool.tile([NP, 6, IW], f32, tag="cm", name="cm")
        Cp = cpool.tile([NP, 6, IW], f32, tag="cp", name="cp")
        C0 = w()
        for k1, rwn, Cd in ((-1, "rwm", Cm[:, 1:5, :]), (0, "rwc", wi(C0)),
                            (1, "rwp", Cp[:, 1:5, :])):
            SR = w()
            eng_prod.tensor_tensor(SR[:], x_full, wgt[rwn][:], Alu.mult)
            PS = {}
            for k0, cwn in ((-1, "cwm"), (0, "cwc"), (1, "cwp")):
                PS[k0] = w()
                eng_prod.tensor_tensor(PS[k0][:], SR[:], wgt[cwn][:], Alu.mult)
            # C[i] = PS[-1][i+1] + PS[0][i] + PS[+1][i-1]; col i at offset i+1
            eng_acc.tensor_tensor(Cd, PS[-1][:, :, 2:2 + IW],
                                  PS[0][:, :, 1:1 + IW], Alu.add)
            eng_acc.tensor_tensor(Cd, Cd, PS[1][:, :, 0:IW], Alu.add)
        # row-halo blocks via DMA
        nc.sync.dma_start(out=Cm[0:NP - 1, 5:6, :], in_=Cm[1:NP, 1:2, :])
        nc.sync.dma_start(out=Cm[NP - 1:NP, 5:6, :], in_=Cm[0:1, 1:2, :])
        nc.sync.dma_start(out=Cp[1:NP, 0:1, :], in_=Cp[0:NP - 1, 4:5, :])
        nc.sync.dma_start(out=Cp[0:1, 0:1, :], in_=Cp[NP - 1:NP, 4:5, :])
        # out[j] = Cm[j+1] + C0[j] + Cp[j-1]
        eng_acc.tensor_tensor(out_int, Cm[:, 2:6, :], wi(C0), Alu.add)
        eng_acc.tensor_tensor(out_int, out_int, Cp[:, 0:4, :], Alu.add)

    # ---------------- init ----------------
    orig_b = w()
    nc.sync.dma_start(
        out=wi(orig_b), in_=ins["orig"].rearrange("(p m) c -> p m c", p=NP))
    inp_b = w()
    nc.sync.dma_start(
        out=wi(inp_b), in_=ins["inp"].rearrange("(p m) c -> p m c", p=NP))
    t0 = w()
    TSS(wi(t0), wi(inp_b), sc["ALPHA"], Alu.mult)
    STT(T_int, wi(orig_b), 1.0 - sc["ALPHA"], wi(t0), Alu.mult, Alu.add)
    TS2(T_int, T_int, 0.5, 0.5, Alu.mult, Alu.add)
    halo_T()
    nc.vector.memset(S[:], 0.0)
    nc.vector.memset(Wt[:], 0.0)
    nc.vector.memset(VS[:], 0.0)

    rain_r = ins["rain"].rearrange("t (p m) c -> t p m c", p=NP)

    # ---------------- iterations ----------------
    for t in range(iters):
        rain_b = rain_pool.tile([NP, 4, IW], f32, tag="rain", name="rain_b")
        nc.sync.dma_start(out=rain_b[:], in_=rain_r[t])

        # gradient (T blocks: data at 1..4; row offset a -> blocks 1+a..5+a)
        DyR = w()
        TT(wi(DyR), T[:, 0:4, 2:2 + IW], T[:, 2:6, 2:2 + IW], Alu.subtract)
        DxR = w()
        TT(wi(DxR), T[:, 1:5, 1:1 + IW], T[:, 1:5, 3:3 + IW], Alu.subtract)
        sqy = w()
        ACT(wi(sqy), wi(DyR), Af.Square, scale=0.5)
        sqx = w()
        ACT(wi(sqx), wi(DxR), Af.Square, scale=0.5)
        s2 = w()
        STT(wi(s2), wi(sqx), 1e-30, wi(sqy), Alu.max, Alu.add)
        lns = w()
        ACT(wi(lns), wi(s2), Af.Ln)
        mag = w()
        ACT(wi(mag), wi(lns), Af.Exp, scale=0.5)
        lnm = w()
        ACT(wi(lnm), wi(mag), Af.Ln, bias=EPS)
        rc = w()
        ACT(wi(rc), wi(lnm), Af.Exp, scale=-1.0)
        gx = w()
        STT(wi(gx), wi(DxR), 0.5, wi(rc), Alu.mult, Alu.mult)
        gy = w()
        STT(wi(gy), wi(DyR), 0.5, wi(rc), Alu.mult, Alu.mult)
        # u = gy drives column weights, v = gx drives row weights (the
        # reference swaps gradient components before sampling/displacing)
        for u_t, pre in ((gy, "c"), (gx, "r")):
            m_i = wgt[pre + "wm"][:, :, 1:1 + IW]
            p_i = wgt[pre + "wp"][:, :, 1:1 + IW]
            c_i = wgt[pre + "wc"][:, :, 1:1 + IW]
            ACT(m_i, wi(u_t), Af.Relu, scale=-1.0)
            ACT(p_i, wi(u_t), Af.Relu)
            tw_ = w()
            TT(wi(tw_), m_i, p_i, Alu.add)
            TS2(c_i, wi(tw_), -1.0, 1.0, Alu.mult, Alu.add)
        for nm in ("cwm", "cwc", "cwp", "rwm", "rwc", "rwp"):
            halo2(wgt[nm])

        # gather: nb = sum_a rw[a] * (sum_b cw[b] * T[j+a, i+b])
        nb = w()
        first_a = True
        for rwn, a in (("rwm", -1), ("rwc", 0), ("rwp", 1)):
            G = w()
            tmp = w()
            tmp2 = w()
            TT(wi(G), wgt["cwm"][:, :, 1:1 + IW],
               T[:, 1 + a:5 + a, 1:1 + IW], Alu.mult)
            TT(wi(tmp), wgt["cwc"][:, :, 1:1 + IW],
               T[:, 1 + a:5 + a, 2:2 + IW], Alu.mult)
            TT(wi(G), wi(G), wi(tmp), Alu.add)
            TT(wi(tmp2), wgt["cwp"][:, :, 1:1 + IW],
               T[:, 1 + a:5 + a, 3:3 + IW], Alu.mult)
            TT(wi(G), wi(G), wi(tmp2), Alu.add)
            if first_a:
                TT(wi(nb), wgt[rwn][:, :, 1:1 + IW], wi(G), Alu.mult)
                first_a = False
            else:
                TT(wi(tmp), wgt[rwn][:, :, 1:1 + IW], wi(G), Alu.mult)
                TT(wi(nb), wi(nb), wi(tmp), Alu.add)
        hd = w()
        TT(wi(hd), T_int, wi(nb), Alu.subtract)

        # velocity (carry VS = velocity^2; V = exp(0.5 ln VS))
        vsn = w()
        STT(wi(vsn), wi(hd), sc["GR"], VS_int, Alu.mult, Alu.add)
        rz5 = w()
        ACT(wi(rz5), wi(vsn), Af.Relu, bias=-EPS)
        t5 = w()
        ACT(wi(t5), wi(vsn), Af.Relu, scale=-1.0, bias=EPS)
        m5 = w()
        ACT(wi(m5), wi(t5), Af.Exp, scale=-1.0, bias=-8.0)
        STT(VS_int, wi(m5), EPS, wi(rz5), Alu.add, Alu.add)
        lnv = w()
        ACT(wi(lnv), VS_int, Af.Ln)
        vel = w()
        ACT(wi(vel), wi(lnv), Af.Exp, scale=0.5)

        # water += rain * 2^rain_rate   (rain >= 0 so the relu is identity;
        # deferred to here so the single-buffered rain DMA hides)
        STT(Wt_int, rain_b[:], sc["RR"], Wt_int, Alu.mult, Alu.add)
        halo2(Wt)

        # new_hd = soft_floor(hd, MHD)
        rz6 = w()
        ACT(wi(rz6), wi(hd), Af.Relu, bias=-sc["MHD"])
        t6 = w()
        ACT(wi(t6), wi(hd), Af.Relu, scale=-1.0, bias=sc["MHD"])
        m6 = w()
        ACT(wi(m6), wi(t6), Af.Exp, scale=-1.0, bias=-8.0)
        nhd = w()
        STT(wi(nhd), wi(m6), sc["MHD"], wi(rz6), Alu.add, Alu.add)

        # sediment capacity
        t7 = w()
        TT(wi(t7), wi(nhd), wi(vel), Alu.mult)
        scap = w()
        STT(wi(scap), wi(t7), sc["SCC"], Wt_int, Alu.mult, Alu.mult)

        # branch coefficients
        ftb = w()
        TSS(wi(ftb), wi(hd), 0.0, Alu.is_lt)
        stb = w()
        TSS(wi(stb), wi(hd), sc["MX"], Alu.is_gt)
        fs = w()
        TT(wi(fs), wi(ftb), wi(stb), Alu.add)
        coef = w()
        TS2(wi(coef), wi(fs), -1.0, 1.0, Alu.mult, Alu.add)
        t9 = w()
        TS2(wi(t9), wi(hd), -1.0, sc["MX"], Alu.mult, Alu.add)
        second = w()
        TT(wi(second), wi(stb), wi(t9), Alu.mult)

        # first
        mint = w()
        ACT(wi(mint), wi(hd), Af.Relu, scale=-1.0)
        z3 = w()
        TT(wi(z3), wi(mint), S_int, Alu.subtract)
        rz3 = w()
        ACT(wi(rz3), wi(z3), Af.Relu)
        t8 = w()
        ACT(wi(t8), wi(z3), Af.Relu, scale=-1.0)
        m8 = w()
        ACT(wi(m8), wi(t8), Af.Exp, scale=-1.0, bias=-8.0)
        q = w()
        TT(wi(q), wi(rz3), wi(m8), Alu.add)
        first = w()
        TT(wi(first), wi(mint), wi(q), Alu.subtract)

        # third
        sdiff = w()
        TT(wi(sdiff), S_int, wi(scap), Alu.subtract)
        r1 = w()
        ACT(wi(r1), wi(sdiff), Af.Relu, scale=sc["DEP"])
        r2 = w()
        ACT(wi(r2), wi(sdiff), Af.Relu, scale=-sc["DIS"])
        t10 = w()
        TT(wi(t10), wi(r1), wi(r2), Alu.subtract)
        third = w()
        TT(wi(third), wi(coef), wi(t10), Alu.mult)

        # deposited = soft_floor(first+second+third, -relu(hd))
        m4 = w()
        ACT(wi(m4), wi(hd), Af.Relu)
        x4 = w()
        TT(wi(x4), wi(first), wi(second), Alu.add)
        TT(wi(x4), wi(x4), wi(third), Alu.add)
        z4 = w()
        TT(wi(z4), wi(x4), wi(m4), Alu.add)
        rz4 = w()
        ACT(wi(rz4), wi(z4), Af.Relu)
        t11 = w()
        ACT(wi(t11), wi(z4), Af.Relu, scale=-1.0)
        m11 = w()
        ACT(wi(m11), wi(t11), Af.Exp, scale=-1.0, bias=-8.0)
        dep0 = w()
        TT(wi(dep0), wi(rz4), wi(m11), Alu.add)
        depo = w()
        TT(wi(depo), wi(dep0), wi(m4), Alu.subtract)

        # state updates
        TT(S_int, S_int, wi(depo), Alu.subtract)
        halo2(S)
        TT(T_int, T_int, wi(depo), Alu.add)
        halo_T()

        # displace sediment, then water (water pre-scaled by 1-2^evap).
        # Water displace runs fully on GPSIMD (its result is only needed
        # next iteration); sediment products on DVE, accumulates on GPSIMD.
        displace(S[:], S_int, nc.vector, nc.gpsimd, "cs")
        Wtk = w()
        TSS(Wtk[:], Wt[:], sc["EV"], Alu.mult)
        displace(Wtk[:], Wt_int, nc.vector, nc.gpsimd, "cw")

    # ---------------- output ----------------
    ob = w()
    TS2(wi(ob), T_int, 2.0, -1.0, Alu.mult, Alu.add)
    nc.sync.dma_start(
        out=outs["out"].rearrange("(p m) c -> p m c", p=NP), in_=wi(ob))


@functools.lru_cache(maxsize=2)
def _compiled(scalar_key, wg, iters):
    from contextlib import ExitStack

    import concourse.tile as tile
    from concourse import bacc, mybir

    sc = dict(scalar_key)
    nc = bacc.Bacc("TRN2", target_bir_lowering=False, debug=False,
                   num_devices=N_CORES)
    f32 = mybir.dt.float32
    # Pre-register const APs for every activation bias value we use.
    for i, v in enumerate([EPS, -EPS, EPS + 8.0, 8.0, -sc["MHD"],
                           sc["MHD"] + 8.0, sc["MHD"], -8.0]):
        v = float(v)
        if (f32, v) not in nc.const_aps.aps:
            ct = nc.alloc_sbuf_tensor(f"constf32_{i}", [128, 1], f32)
            nc.gpsimd.memset(ct.ap(), v)
            nc.const_aps.aps[(f32, v)] = ct.ap()
    nc.all_engine_barrier()
    # Force every activation function into the one table set that contains
    # them all (natural_log_exp_and_others: exp, ln, relu, square, ...) so
    # the compiler never inserts mid-kernel ACT_TABLE_LOAD switches.
    try:
        from concourse.hw_specs import get_activation_tables

        tbl = get_activation_tables(nc.m.arch)
        keep = {mybir.ActivationFunctionType.Exp, mybir.ActivationFunctionType.Ln,
                mybir.ActivationFunctionType.Relu,
                mybir.ActivationFunctionType.Square}
        if "natural_log_exp_and_others" in tbl and keep <= tbl[
                "natural_log_exp_and_others"]:
            for name, fns in tbl.items():
                if name != "natural_log_exp_and_others":
                    fns -= keep
    except Exception:
        pass
    inp = nc.dram_tensor("inp", [wg, wg], f32, kind="ExternalInput")
    orig = nc.dram_tensor("orig", [wg, wg], f32, kind="ExternalInput")
    rain = nc.dram_tensor("rain", [iters, wg, wg], f32, kind="ExternalInput")
    out = nc.dram_tensor("out", [wg, wg], f32, kind="ExternalOutput")
    ins = {"inp": inp.ap(), "orig": orig.ap(), "rain": rain.ap()}
    outs = {"out": out.ap()}
    with ExitStack() as ctx:
        tc = ctx.enter_context(tile.TileContext(nc))
        build_erosion(nc, tc, ctx, ins, outs, sc, wg, iters)
    nc.compile()
    return nc


def kernel(**inputs):
    from concourse.bass_utils import run_bass_kernel_spmd

    it = np.ascontiguousarray(np.asarray(inputs["input_terrain"], np.float32))
    ot = np.ascontiguousarray(np.asarray(inputs["original_terrain"], np.float32))
    rain = np.ascontiguousarray(
        np.asarray(inputs["random_rainfall"], np.float32)[0])  # [ITERS, W, W]
    sc = _scalars(
        inputs["rain_rate"], inputs["evaporation_rate"],
        inputs["min_height_delta"], inputs["gravity"],
        inputs["sediment_capacity_constant"], inputs["dissolving_rate"],
        inputs["deposition_rate"], inputs["max_height_delta"],
        inputs["alpha"], W)
    nc = _compiled(tuple(sorted(sc.items())), W, ITERS)
    in_maps = [
        {"inp": it[c], "orig": ot[c], "rain": rain} for c in range(B)
    ]
    res = run_bass_kernel_spmd(nc, in_maps, core_ids=list(range(N_CORES)))
    out = np.stack([res.results[c]["out"] for c in range(B)])[:, None]
    return out.astype(np.float32)


if __name__ == "__main__":
    # smoke build
    sc = _scalars(-6.0388, -5.643, -10.965, 4.906, 5.643, -2.0, -4.321,
                  -8.965, 0.0, W)
    nc = _compiled(tuple(sorted(sc.items())), W, ITERS)
    print("built ok:",
          sum(len(b.instructions) for b in nc.main_func.blocks), "instructions")
